# revision 23
# baseline (speedup 1.0000x reference)
"""MixerHead kernel for 8 trn2 NeuronCores (Bass/Tile, bf16 matmuls).

Math (reference):
  proj[b,h,l,e]  = sum_d x[b,l,d] Wp[h,e,d] + bp[h,e]
  mixed[b,h,f,e] = sum_{l<=f} Wc[h,f,l] proj[b,h,l,e] + bc[h,f]
  out[b,f,j]     = sum_{h,e} mixed[b,h,f,e] Wo[j, h*E+e] + bo[j]

Sharding: core c = (batch b = c//2, head-pair hp = c%2 -> heads {2hp, 2hp+1}).
Each core computes the bias-free linear part for its (batch, 2 heads) and
writes a partial [L, D] output; host sums the two partials per batch and adds
all bias contributions (folded into a single [L, D] matrix analytically).

Device layout chain (every matmul is out = lhsT.T @ rhs, contraction on the
partition dim):
  phase1: proj[l,e]    lhsT = xT[d, l-tile]          rhs = WpT[d, e(512)]
  phase2: mixedT[e,f]  lhsT = proj[l-tile, e-block]  rhs = WcT[l-tile, f-cols]
          exact 128-block causal staircase: off-diagonal l-tiles use the full
          512 f-cols; the 4 diagonal l-tiles j use narrowing rhs widths
          512-128j written at psum column offset 128j (the first full-width
          matmul carries start=True, so every psum element's first write
          overwrites; later narrower ones accumulate).
  phase3: part[f,dout] lhsT = mixedT[e-blk, f-tile]  rhs = WoT[e-blk, dout]

Schedule notes (all measured against perfetto/NTFF traces):
- DMA rings all round-robin over the same 16 HW DMA engines, so priority
  only exists as ordering within a ring. The sync ring carries every input
  except xt0 in strict global need-order (wp, wc0, xt1, wc1, xt2, wo, wc2,
  xt3, wc3); xt0 rides the scalar ring so the cold window streams wp+xt0
  concurrently; out DMAs ride the otherwise-idle gpsimd ring.
- PSUM plumbing: 2 banks for phase1 (i-outer chunks only ever hold 2),
  2 for phase2, and a 4-deep ring (b0-b3) for phase3 so psum-release casts
  are never the limiter; chunk 0 borrows b0/b1 (phase1 d-outer needs 4
  accumulators) and b2/b3 (phase2's 852ns groups) before p3.0 needs them.
- Copy-engine FIFOs are ordered to match PE need order: proj casts on
  scalar for chunks >=1, mix copies split eb0/1->vector eb2/3->scalar,
  out casts dc0->vector dc1->scalar; the final piece is split across both
  psum readers and DMA'd as 64KB quarters from two rings to cut the drain.
- 8 warmup matmuls (~3.4us of PE busy) ramp the HAM clock gate during the
  cold-start DMA window.
"""

import sys

for _p in ("/opt/trn_rl_repo", "/root/.axon_site/_ro/trn_rl_repo"):
    if _p not in sys.path:
        sys.path.append(_p)

import numpy as np

import ml_dtypes

try:  # make trace requests degrade gracefully if the NTFF hook module is absent
    import antenv.axon_hooks  # noqa: F401
except ImportError:
    import types

    import antenv

    _m = types.ModuleType("antenv.axon_hooks")
    _h = {}
    _m.set_axon_ntff_profile_hook = lambda hook: _h.__setitem__("h", hook)
    _m.get_axon_ntff_profile_hook = lambda: _h.get("h")
    sys.modules["antenv.axon_hooks"] = _m
    antenv.axon_hooks = _m

from concourse import bacc, mybir, tile
from concourse.bass_utils import run_bass_kernel_spmd

B, L, D, H, E = 4, 2048, 1024, 4, 256
F32 = mybir.dt.float32
BF16 = mybir.dt.bfloat16

LT = L // 128   # 16 l-tiles per batch
FC = 4          # f-chunks of 512
DT8 = D // 128  # 8 d-tiles
LT4 = 4         # l-chunks of 512

# Packed Wc layout (per head): per f-chunk c, off-diagonal l-tiles 0..4c-1 at
# full 512 f-cols, then diagonal l-tiles j=0..3 at 512-128j f-cols.
WC_CHUNK_W = [4 * c * 512 + 1280 for c in range(FC)]     # 1280,3328,5376,7424
WC_PACK_COLS = sum(WC_CHUNK_W)                            # 17408
DIAG_OFF = [0, 512, 896, 1152]                            # col offset of diag j
DIAG_N = [512, 384, 256, 128]

# Set by test harness: run with trace and record exec time.
TRACE = False
LAST_EXEC_NS = None

_cache = {}


def _build_program():
    if "nc" in _cache:
        return _cache["nc"]
    nc = bacc.Bacc("TRN2", target_bir_lowering=False, debug=False, num_devices=8)

    # Pre-tiled (partition-major) input layouts: every DMA below is a plain
    # 2D slice with long contiguous per-partition lines (1-8KB), so the HW
    # DMA engines run near their per-packet peak during the cold window.
    xp = nc.dram_tensor("xp", [128, LT4 * 4096], BF16, kind="ExternalInput")
    wpp = nc.dram_tensor("wpp", [128, DT8 * 512], BF16, kind="ExternalInput")
    wc0 = nc.dram_tensor("wc0", [128, WC_PACK_COLS], BF16, kind="ExternalInput")
    wc1 = nc.dram_tensor("wc1", [128, WC_PACK_COLS], BF16, kind="ExternalInput")
    wop = nc.dram_tensor("wop", [128, 4 * D], BF16, kind="ExternalInput")
    part = nc.dram_tensor("part", [L, D], BF16, kind="ExternalOutput")
    wc_dram = [wc0, wc1]

    with tile.TileContext(nc) as tc:
        with (
            tc.tile_pool(name="wp", bufs=1) as wp_pool,
            tc.tile_pool(name="wo", bufs=1) as wo_pool,
            tc.tile_pool(name="xt", bufs=1) as x_pool,
            tc.tile_pool(name="wc", bufs=1) as wc_pool,
            tc.tile_pool(name="proj", bufs=1) as proj_pool,
            tc.tile_pool(name="mix", bufs=1) as mix_pool,
            tc.tile_pool(name="outs", bufs=8) as out_pool,
            tc.tile_pool(name="psA", bufs=1, space="PSUM") as psA_pool,
            tc.tile_pool(name="ps2", bufs=2, space="PSUM") as ps2_pool,
            tc.tile_pool(name="psB", bufs=1, space="PSUM") as psB_pool,
        ):
            # PE warm-up: dummy matmuls with no DMA dependency run during the
            # startup loads so the HAM clock ramps before the first real
            # matmul. The memset goes on vector (idle at start) so it does
            # not delay any DMA-issuing engine.
            warm = wp_pool.tile([128, 512], BF16, tag="warm")
            nc.vector.memset(warm[:], 0.0)
            ps_w = psB_pool.tile([128, 512], F32, tag="b2", name="ps_warm")
            for _ in range(7):
                nc.tensor.matmul(
                    ps_w[:], warm[:, :128], warm[:], start=True, stop=True
                )

            # ---- input DMAs ----
            # All DMA rings round-robin over the same 16 HW engines, so
            # "priority" only exists as ordering WITHIN one ring. The cold
            # window interleaves the (xt0-d, wp-d) pieces phase1.0 consumes
            # first across the sync+vector rings (scalar is blocked ~1.3us
            # at start by its ACT_TABLE_LOAD) so the first matmul unblocks
            # as early as possible; the rest rides sync in global need-order.
            wp_all = wp_pool.tile([128, DT8 * 512], BF16, tag="wp")
            xt_tiles = {}
            xt0 = x_pool.tile([128, DT8 * 512], BF16, tag="xt0", name="xt_0")
            xt_tiles[0] = xt0

            def xs(c, a, b):
                return xp[:, c * 4096 + a : c * 4096 + b]

            # Early DMA: per-queue rate is ~100-135GB/s and queues add (the
            # HW engines round-robin over every core's queues), so the cold
            # stream uses TWO queues, each in strict consumption order:
            # the xt pieces (LDW operand) on sync, the wp pieces (rhs) on
            # gpsimd, graduated piece sizes.
            # three cold queues: sync (fast start) takes d0-d1 + xt-d2d3,
            # gpsimd takes wp-d1..d5 + xt-d4d5, scalar (starts ~9.9 after
            # its ACT_TABLE_LOAD) takes the late d6-d7 pieces.
            nc.sync.dma_start(xt0[:, 0:512], xs(0, 0, 512))
            nc.sync.dma_start(wp_all[:, 0:512], wpp[:, 0:512])
            nc.gpsimd.dma_start(wp_all[:, 512:1024], wpp[:, 512:1024])
            nc.sync.dma_start(xt0[:, 512:1024], xs(0, 512, 1024))
            nc.gpsimd.dma_start(wp_all[:, 1024:2048], wpp[:, 1024:2048])
            nc.sync.dma_start(xt0[:, 1024:2048], xs(0, 1024, 2048))
            nc.gpsimd.dma_start(xt0[:, 2048:3072], xs(0, 2048, 3072))
            nc.scalar.dma_start(wp_all[:, 2048:3072], wpp[:, 2048:3072])
            nc.scalar.dma_start(xt0[:, 3072:4096], xs(0, 3072, 4096))
            nc.gpsimd.dma_start(wp_all[:, 3072:4096], wpp[:, 3072:4096])
            wp = [wp_all[:, d * 512 : (d + 1) * 512] for d in range(DT8)]

            def load_xt(c):
                xt_all = x_pool.tile(
                    [128, DT8 * 512], BF16, tag=f"xt{c}", name=f"xt_{c}"
                )
                xt_tiles[c] = xt_all
                for g in range(2):
                    nc.sync.dma_start(
                        xt_all[:, g * 2048 : (g + 1) * 2048],
                        xs(c, g * 2048, (g + 1) * 2048),
                    )

            wc_sb = [[None] * FC for _ in range(2)]

            def load_wc(c):
                for hh in range(2):
                    wct = wc_pool.tile(
                        [128, WC_CHUNK_W[c]], BF16, tag=f"wc{hh}_{c}",
                        name=f"wc_{hh}_{c}",
                    )
                    off = sum(WC_CHUNK_W[:c])
                    nc.sync.dma_start(
                        wct[:], wc_dram[hh][:, off : off + WC_CHUNK_W[c]]
                    )
                    wc_sb[hh][c] = wct

            wo_all = wo_pool.tile([128, 4 * D], BF16, tag="wo", name="wo_all")

            # sync ring, global need-order after the cold window:
            load_wc(0)
            load_xt(1)
            load_wc(1)
            load_xt(2)
            nc.sync.dma_start(wo_all[:], wop[:, :])
            load_wc(2)
            load_xt(3)
            load_wc(3)

            proj = [None] * LT
            mix = [[None] * FC for _ in range(4)]

            def cast_proj(c, i, eng):
                lt = c * 4 + i
                pt = proj_pool.tile(
                    [128, 2 * E], BF16, tag=f"proj{lt}", name=f"proj_{lt}"
                )
                eng(pt[:], ps1_tiles[i][:])
                proj[lt] = pt

            ps1_tiles = [None] * 4

            def phase1(c):
                xt_all = xt_tiles[c]
                # chunk 0 (d-outer) accumulates 4 psums at once: 2 from the
                # phase1 pool + 2 borrowed from phase3's ring (free until
                # p3.0). Chunks >=1 (i-outer) only ever hold 2.
                for i in range(4):
                    if c == 0 and i >= 2:
                        ps1_tiles[i] = psB_pool.tile(
                            [128, 2 * E], F32, tag=f"b{i - 2}", name=f"ps1_{c}_{i}"
                        )
                    else:
                        ps1_tiles[i] = psA_pool.tile(
                            [128, 2 * E], F32, tag=f"a{i % 2}", name=f"ps1_{c}_{i}"
                        )
                if c == 0:
                    # d-outer: consume wp/xt pieces at their arrival pace
                    for d in range(DT8):
                        for i in range(4):
                            nc.tensor.matmul(
                                ps1_tiles[i][:],
                                xt_all[:, d * 512 + i * 128 : d * 512 + (i + 1) * 128],
                                wp[d],
                                start=(d == 0),
                                stop=(d == DT8 - 1),
                            )
                    for i in range(4):
                        cast_proj(c, i, nc.vector.tensor_copy if i % 2 == 0 else nc.scalar.copy)
                else:
                    # i-outer: each proj tile finishes (and casts) early.
                    # Casts all on scalar: they'd otherwise queue ahead of
                    # p3(c-1)'s dc0 psum-release casts in vector's FIFO.
                    for i in range(4):
                        for d in range(DT8):
                            nc.tensor.matmul(
                                ps1_tiles[i][:],
                                xt_all[:, d * 512 + i * 128 : d * 512 + (i + 1) * 128],
                                wp[d],
                                start=(d == 0),
                                stop=(d == DT8 - 1),
                            )
                        cast_proj(c, i, nc.scalar.copy)

            def phase2(c):
                # exact causal staircase (128-block granularity via narrowing N)
                for eb in range(4):
                    wct = wc_sb[eb // 2][c]
                    # chunk 0's short (852ns) groups would stall on the
                    # 2-deep ps2 ring waiting for mix copies; borrow the two
                    # b-banks that are idle until p3.0.
                    if c == 0 and eb >= 2:
                        ps = psB_pool.tile(
                            [128, 512], F32, tag=f"b{eb}", name=f"ps2_{c}_{eb}"
                        )
                    else:
                        ps = ps2_pool.tile(
                            [128, 512], F32, tag="ps2", name=f"ps2_{c}_{eb}"
                        )
                    for t in range(4 * c):
                        nc.tensor.matmul(
                            ps[:],
                            proj[t][:, eb * 128 : (eb + 1) * 128],
                            wct[:, t * 512 : (t + 1) * 512],
                            start=(t == 0),
                            stop=False,
                        )
                    for j in range(4):
                        n = DIAG_N[j]
                        col = 4 * c * 512 + DIAG_OFF[j]
                        nc.tensor.matmul(
                            ps[:, 128 * j : 512],
                            proj[4 * c + j][:, eb * 128 : (eb + 1) * 128],
                            wct[:, col : col + n],
                            start=(c == 0 and j == 0),
                            stop=(j == 3),
                        )
                    mt = mix_pool.tile(
                        [128, 512], BF16, tag=f"m{eb}_{c}", name=f"mix_{eb}_{c}"
                    )
                    if eb < 2:
                        nc.vector.tensor_copy(mt[:], ps[:])
                    else:
                        nc.scalar.copy(mt[:], ps[:])
                    mix[eb][c] = mt

            def phase3(c):
                for fi in range(4):
                    ft = c * 4 + fi
                    ot = out_pool.tile(
                        [128, D], BF16, tag="out", name=f"out_{ft}"
                    )
                    for dc in range(2):
                        g = 2 * fi + dc
                        ps = psB_pool.tile(
                            [128, 512], F32, tag=f"b{g % 4}", name=f"ps3_{ft}_{dc}"
                        )
                        for eb in range(4):
                            nc.tensor.matmul(
                                ps[:],
                                mix[eb][c][:, fi * 128 : (fi + 1) * 128],
                                wo_all[
                                    :, eb * D + dc * 512 : eb * D + (dc + 1) * 512
                                ],
                                start=(eb == 0),
                                stop=(eb == 3),
                            )
                        osl = ot[:, dc * 512 : (dc + 1) * 512]
                        last = c == FC - 1 and fi == 3 and dc == 1
                        if last:
                            # final cast all on vector: scalar's dispatch
                            # lags ~0.5us behind the last matmul here
                            nc.vector.tensor_copy(osl, ps[:])
                        else:
                            if dc == 0:
                                nc.vector.tensor_copy(osl, ps[:])
                            else:
                                nc.scalar.copy(osl, ps[:])
                            if c == FC - 1 and fi != 3:
                                # last chunk avoids gpsimd entirely: its
                                # SWDGE drain costs ~3us when active at the
                                # end. dc0 on sync (idle by now), dc1 scalar.
                                deng = nc.sync if dc == 0 else nc.scalar
                                deng.dma_start(
                                    part[
                                        ft * 128 : (ft + 1) * 128,
                                        dc * 512 : (dc + 1) * 512,
                                    ],
                                    osl,
                                )
                    if c != FC - 1:
                        nc.gpsimd.dma_start(
                            part[ft * 128 : (ft + 1) * 128, :], ot[:]
                        )
                    elif fi == 3:
                        # ft15: row-halves (2KB contiguous lines) on the two
                        # drain rings -- one DMA instruction per queue
                        nc.sync.dma_start(
                            part[ft * 128 : ft * 128 + 64, :], ot[:64, :]
                        )
                        nc.scalar.dma_start(
                            part[ft * 128 + 64 : (ft + 1) * 128, :], ot[64:, :]
                        )

            phase1(0)
            phase2(0)
            for c in range(1, FC):
                phase1(c)
                phase3(c - 1)
                phase2(c)
            phase3(FC - 1)

    nc.compile()
    _cache["nc"] = nc
    return nc


def _pack_wc_head(wc_h: np.ndarray) -> np.ndarray:
    """tril(Wc[h]) -> [128, 17408]: per f-chunk c, full off-diagonal l-tiles
    0..4c-1 (512 f-cols each) followed by diagonal l-tiles j=0..3 at
    narrowing widths 512-128j (cols 128j..512 of the chunk)."""
    m = np.tril(wc_h)  # [f, l]
    cols = []
    for c in range(FC):
        if c:
            sub = m[c * 512 : (c + 1) * 512, : 4 * c * 128]  # [512 f, 4c*128 l]
            subT = np.ascontiguousarray(sub.T).reshape(4 * c, 128, 512)
            cols.append(subT.transpose(1, 0, 2).reshape(128, 4 * c * 512))
        for j in range(4):
            blk = m[
                c * 512 + 128 * j : (c + 1) * 512,
                (4 * c + j) * 128 : (4 * c + j + 1) * 128,
            ]  # [512-128j f, 128 l]
            cols.append(np.ascontiguousarray(blk.T))
    return np.ascontiguousarray(np.concatenate(cols, axis=1)).astype(
        ml_dtypes.bfloat16
    )


def kernel(x, Wp, bp, Wc, bc, Wo, bo):
    global LAST_EXEC_NS
    x = np.asarray(x, dtype=np.float32)
    Wp = np.asarray(Wp, dtype=np.float32)
    bp = np.asarray(bp, dtype=np.float32)
    Wc = np.asarray(Wc, dtype=np.float32)
    bc = np.asarray(bc, dtype=np.float32)
    Wo = np.asarray(Wo, dtype=np.float32)
    bo = np.asarray(bo, dtype=np.float32)

    nc = _build_program()

    WoT = np.ascontiguousarray(Wo.T)  # [din, dout]
    wc_packed = [_pack_wc_head(Wc[h]) for h in range(H)]
    # Pre-tiled (partition-major) layouts matching the SBUF tiles exactly:
    #   x_pre[b][p, c*4096 + d*512 + l] = x[b, c*512+l, d*128+p]
    #   wp_pre[hp][p, d*512 + e]        = WpT_pair[d*128+p, e]
    #   wo_pre[hp][p, eb*1024 + j]      = WoT_pair[eb*128+p, j]
    x_pre = [
        np.ascontiguousarray(
            x[b].reshape(4, 512, 8, 128).transpose(3, 0, 2, 1).reshape(128, 16384)
        ).astype(ml_dtypes.bfloat16)
        for b in range(B)
    ]
    wp_pre = []
    wo_pre = []
    for hp in range(2):
        h0, h1 = 2 * hp, 2 * hp + 1
        wpT_pair = np.concatenate([Wp[h0].T, Wp[h1].T], axis=1)  # [D, 2E]
        wp_pre.append(
            np.ascontiguousarray(
                wpT_pair.reshape(8, 128, 512).transpose(1, 0, 2).reshape(128, 4096)
            ).astype(ml_dtypes.bfloat16)
        )
        woT_pair = np.concatenate(
            [WoT[h0 * E : (h0 + 1) * E], WoT[h1 * E : (h1 + 1) * E]], axis=0
        )  # [2E, D]
        wo_pre.append(
            np.ascontiguousarray(
                woT_pair.reshape(4, 128, 1024).transpose(1, 0, 2).reshape(128, 4096)
            ).astype(ml_dtypes.bfloat16)
        )

    in_maps = []
    for c in range(8):
        b, hp = c // 2, c % 2
        in_maps.append(
            {
                "xp": x_pre[b],
                "wpp": wp_pre[hp],
                "wc0": wc_packed[2 * hp],
                "wc1": wc_packed[2 * hp + 1],
                "wop": wo_pre[hp],
            }
        )

    res = run_bass_kernel_spmd(
        nc, in_maps, core_ids=list(range(8)), trace=TRACE
    )
    LAST_EXEC_NS = res.exec_time_ns

    # Host: fold all bias terms into one [L, D] matrix.
    # mixed bias = tril-rowsum(Wc)[h,f] * bp[h,e] + bc[h,f]; through Wo:
    rs = np.tril(Wc).sum(axis=2)  # [H, L]
    Wo_hE = Wo.reshape(D, H, E)
    V = np.einsum("he,jhe->hj", bp, Wo_hE)  # [H, D]
    WoSum = Wo_hE.sum(axis=2)  # [D, H]
    bias_total = rs.T @ V + bc.T @ WoSum.T + bo[None, :]  # [L, D]

    out = np.empty((B, L, D), dtype=np.float32)
    for b in range(B):
        out[b] = (
            res.results[2 * b]["part"].astype(np.float32)
            + res.results[2 * b + 1]["part"].astype(np.float32)
            + bias_total
        )
    return out



# revision 25
# speedup vs baseline: 1.0354x; 1.0354x over previous
"""MixerHead kernel for 8 trn2 NeuronCores (Bass/Tile, bf16 matmuls).

Math (reference):
  proj[b,h,l,e]  = sum_d x[b,l,d] Wp[h,e,d] + bp[h,e]
  mixed[b,h,f,e] = sum_{l<=f} Wc[h,f,l] proj[b,h,l,e] + bc[h,f]
  out[b,f,j]     = sum_{h,e} mixed[b,h,f,e] Wo[j, h*E+e] + bo[j]

Sharding: core c = (batch b = c//2, head-pair hp = c%2 -> heads {2hp, 2hp+1}).
Each core computes the bias-free linear part for its (batch, 2 heads) and
writes a partial [L, D] output; host sums the two partials per batch and adds
all bias contributions (folded into a single [L, D] matrix analytically).

Device layout chain (every matmul is out = lhsT.T @ rhs, contraction on the
partition dim):
  phase1: proj[l,e]    lhsT = xT[d, l-tile]          rhs = WpT[d, e(512)]
  phase2: mixedT[e,f]  lhsT = proj[l-tile, e-block]  rhs = WcT[l-tile, f-cols]
          exact 128-block causal staircase: off-diagonal l-tiles use the full
          512 f-cols; the 4 diagonal l-tiles j use narrowing rhs widths
          512-128j written at psum column offset 128j (the first full-width
          matmul carries start=True, so every psum element's first write
          overwrites; later narrower ones accumulate).
  phase3: part[f,dout] lhsT = mixedT[e-blk, f-tile]  rhs = WoT[e-blk, dout]

Schedule notes (all measured against perfetto/NTFF traces):
- DMA rings all round-robin over the same 16 HW DMA engines, so priority
  only exists as ordering within a ring. The sync ring carries every input
  except xt0 in strict global need-order (wp, wc0, xt1, wc1, xt2, wo, wc2,
  xt3, wc3); xt0 rides the scalar ring so the cold window streams wp+xt0
  concurrently; out DMAs ride the otherwise-idle gpsimd ring.
- PSUM plumbing: 2 banks for phase1 (i-outer chunks only ever hold 2),
  2 for phase2, and a 4-deep ring (b0-b3) for phase3 so psum-release casts
  are never the limiter; chunk 0 borrows b0/b1 (phase1 d-outer needs 4
  accumulators) and b2/b3 (phase2's 852ns groups) before p3.0 needs them.
- Copy-engine FIFOs are ordered to match PE need order: proj casts on
  scalar for chunks >=1, mix copies split eb0/1->vector eb2/3->scalar,
  out casts dc0->vector dc1->scalar; the final piece is split across both
  psum readers and DMA'd as 64KB quarters from two rings to cut the drain.
- 8 warmup matmuls (~3.4us of PE busy) ramp the HAM clock gate during the
  cold-start DMA window.
"""

import sys

for _p in ("/opt/trn_rl_repo", "/root/.axon_site/_ro/trn_rl_repo"):
    if _p not in sys.path:
        sys.path.append(_p)

import numpy as np

import ml_dtypes

try:  # make trace requests degrade gracefully if the NTFF hook module is absent
    import antenv.axon_hooks  # noqa: F401
except ImportError:
    import types

    import antenv

    _m = types.ModuleType("antenv.axon_hooks")
    _h = {}
    _m.set_axon_ntff_profile_hook = lambda hook: _h.__setitem__("h", hook)
    _m.get_axon_ntff_profile_hook = lambda: _h.get("h")
    sys.modules["antenv.axon_hooks"] = _m
    antenv.axon_hooks = _m

from concourse import bacc, mybir, tile
from concourse.bass_utils import run_bass_kernel_spmd

B, L, D, H, E = 4, 2048, 1024, 4, 256
F32 = mybir.dt.float32
BF16 = mybir.dt.bfloat16

LT = L // 128   # 16 l-tiles per batch
FC = 4          # f-chunks of 512
DT8 = D // 128  # 8 d-tiles
LT4 = 4         # l-chunks of 512

# Packed Wc layout (per head): per f-chunk c, off-diagonal l-tiles 0..4c-1 at
# full 512 f-cols, then diagonal l-tiles j=0..3 at 512-128j f-cols.
WC_CHUNK_W = [4 * c * 512 + 1280 for c in range(FC)]     # 1280,3328,5376,7424
WC_PACK_COLS = sum(WC_CHUNK_W)                            # 17408
DIAG_OFF = [0, 512, 896, 1152]                            # col offset of diag j
DIAG_N = [512, 384, 256, 128]

# Set by test harness: run with trace and record exec time.
TRACE = False
LAST_EXEC_NS = None

_cache = {}


def _build_program():
    if "nc" in _cache:
        return _cache["nc"]
    nc = bacc.Bacc("TRN2", target_bir_lowering=False, debug=False, num_devices=8)

    # Pre-tiled (partition-major) input layouts: every DMA below is a plain
    # 2D slice with long contiguous per-partition lines (1-8KB), so the HW
    # DMA engines run near their per-packet peak during the cold window.
    xp = nc.dram_tensor("xp", [128, LT4 * 4096], BF16, kind="ExternalInput")
    wpp = nc.dram_tensor("wpp", [128, DT8 * 512], BF16, kind="ExternalInput")
    wc0 = nc.dram_tensor("wc0", [128, WC_PACK_COLS], BF16, kind="ExternalInput")
    wc1 = nc.dram_tensor("wc1", [128, WC_PACK_COLS], BF16, kind="ExternalInput")
    wop = nc.dram_tensor("wop", [128, 4 * D], BF16, kind="ExternalInput")
    part = nc.dram_tensor("part", [L, D], BF16, kind="ExternalOutput")
    wc_dram = [wc0, wc1]

    with tile.TileContext(nc) as tc:
        with (
            tc.tile_pool(name="wp", bufs=1) as wp_pool,
            tc.tile_pool(name="wo", bufs=1) as wo_pool,
            tc.tile_pool(name="xt", bufs=1) as x_pool,
            tc.tile_pool(name="wc", bufs=1) as wc_pool,
            tc.tile_pool(name="proj", bufs=1) as proj_pool,
            tc.tile_pool(name="mix", bufs=1) as mix_pool,
            tc.tile_pool(name="outs", bufs=8) as out_pool,
            tc.tile_pool(name="psA", bufs=1, space="PSUM") as psA_pool,
            tc.tile_pool(name="ps2", bufs=2, space="PSUM") as ps2_pool,
            tc.tile_pool(name="psB", bufs=1, space="PSUM") as psB_pool,
        ):
            # PE warm-up: dummy matmuls with no DMA dependency run during the
            # startup loads so the HAM clock ramps before the first real
            # matmul. The memset goes on vector (idle at start) so it does
            # not delay any DMA-issuing engine.
            warm = wp_pool.tile([128, 512], BF16, tag="warm")
            nc.vector.memset(warm[:], 0.0)
            ps_w = psB_pool.tile([128, 512], F32, tag="b2", name="ps_warm")
            for _ in range(8):
                nc.tensor.matmul(
                    ps_w[:], warm[:, :128], warm[:], start=True, stop=True
                )

            # ---- input DMAs ----
            # All DMA rings round-robin over the same 16 HW engines, so
            # "priority" only exists as ordering WITHIN one ring. The cold
            # window interleaves the (xt0-d, wp-d) pieces phase1.0 consumes
            # first across the sync+vector rings (scalar is blocked ~1.3us
            # at start by its ACT_TABLE_LOAD) so the first matmul unblocks
            # as early as possible; the rest rides sync in global need-order.
            wp_all = wp_pool.tile([128, DT8 * 512], BF16, tag="wp")
            xt_tiles = {}
            xt0 = x_pool.tile([128, DT8 * 512], BF16, tag="xt0", name="xt_0")
            xt_tiles[0] = xt0

            def xs(c, a, b):
                return xp[:, c * 4096 + a : c * 4096 + b]

            # Early DMA: per-queue rate is ~100-135GB/s and queues add (the
            # HW engines round-robin over every core's queues), so the cold
            # stream uses TWO queues, each in strict consumption order:
            # the xt pieces (LDW operand) on sync, the wp pieces (rhs) on
            # gpsimd, graduated piece sizes.
            # Proven cold split (measured: sync early rate is only
            # ~65-80GB/s and scalar starts ~1.8us late behind its
            # ACT_TABLE_LOAD, so first-matmul ~11.5us is the physical
            # wall): wp d-pairs on sync, xt0 d-pairs on scalar, both in
            # phase1.0's d-consumption order.
            for g in range(4):
                nc.sync.dma_start(
                    wp_all[:, g * 1024 : (g + 1) * 1024],
                    wpp[:, g * 1024 : (g + 1) * 1024],
                )
            for g in range(4):
                nc.scalar.dma_start(
                    xt0[:, g * 1024 : (g + 1) * 1024],
                    xs(0, g * 1024, (g + 1) * 1024),
                )
            wp = [wp_all[:, d * 512 : (d + 1) * 512] for d in range(DT8)]

            def load_xt(c):
                xt_all = x_pool.tile(
                    [128, DT8 * 512], BF16, tag=f"xt{c}", name=f"xt_{c}"
                )
                xt_tiles[c] = xt_all
                for g in range(2):
                    nc.sync.dma_start(
                        xt_all[:, g * 2048 : (g + 1) * 2048],
                        xs(c, g * 2048, (g + 1) * 2048),
                    )

            wc_sb = [[None] * FC for _ in range(2)]

            def load_wc(c):
                for hh in range(2):
                    wct = wc_pool.tile(
                        [128, WC_CHUNK_W[c]], BF16, tag=f"wc{hh}_{c}",
                        name=f"wc_{hh}_{c}",
                    )
                    off = sum(WC_CHUNK_W[:c])
                    nc.sync.dma_start(
                        wct[:], wc_dram[hh][:, off : off + WC_CHUNK_W[c]]
                    )
                    wc_sb[hh][c] = wct

            wo_all = wo_pool.tile([128, 4 * D], BF16, tag="wo", name="wo_all")

            # sync ring, global need-order after the cold window:
            load_wc(0)
            load_xt(1)
            load_wc(1)
            load_xt(2)
            nc.sync.dma_start(wo_all[:], wop[:, :])
            load_wc(2)
            load_xt(3)
            load_wc(3)

            proj = [None] * LT
            mix = [[None] * FC for _ in range(4)]

            def cast_proj(c, i, eng):
                lt = c * 4 + i
                pt = proj_pool.tile(
                    [128, 2 * E], BF16, tag=f"proj{lt}", name=f"proj_{lt}"
                )
                eng(pt[:], ps1_tiles[i][:])
                proj[lt] = pt

            ps1_tiles = [None] * 4

            def phase1(c):
                xt_all = xt_tiles[c]
                # chunk 0 (d-outer) accumulates 4 psums at once: 2 from the
                # phase1 pool + 2 borrowed from phase3's ring (free until
                # p3.0). Chunks >=1 (i-outer) only ever hold 2.
                for i in range(4):
                    if c == 0 and i >= 2:
                        ps1_tiles[i] = psB_pool.tile(
                            [128, 2 * E], F32, tag=f"b{i - 2}", name=f"ps1_{c}_{i}"
                        )
                    else:
                        ps1_tiles[i] = psA_pool.tile(
                            [128, 2 * E], F32, tag=f"a{i % 2}", name=f"ps1_{c}_{i}"
                        )
                if c == 0:
                    # d-outer: consume wp/xt pieces at their arrival pace
                    for d in range(DT8):
                        for i in range(4):
                            nc.tensor.matmul(
                                ps1_tiles[i][:],
                                xt_all[:, d * 512 + i * 128 : d * 512 + (i + 1) * 128],
                                wp[d],
                                start=(d == 0),
                                stop=(d == DT8 - 1),
                            )
                    for i in range(4):
                        cast_proj(c, i, nc.vector.tensor_copy if i % 2 == 0 else nc.scalar.copy)
                else:
                    # i-outer: each proj tile finishes (and casts) early.
                    # Casts all on scalar: they'd otherwise queue ahead of
                    # p3(c-1)'s dc0 psum-release casts in vector's FIFO.
                    for i in range(4):
                        for d in range(DT8):
                            nc.tensor.matmul(
                                ps1_tiles[i][:],
                                xt_all[:, d * 512 + i * 128 : d * 512 + (i + 1) * 128],
                                wp[d],
                                start=(d == 0),
                                stop=(d == DT8 - 1),
                            )
                        cast_proj(c, i, nc.scalar.copy)

            def phase2(c):
                # exact causal staircase (128-block granularity via narrowing N)
                for eb in range(4):
                    wct = wc_sb[eb // 2][c]
                    # chunk 0's short (852ns) groups would stall on the
                    # 2-deep ps2 ring waiting for mix copies; borrow the two
                    # b-banks that are idle until p3.0.
                    if c == 0 and eb >= 2:
                        ps = psB_pool.tile(
                            [128, 512], F32, tag=f"b{eb}", name=f"ps2_{c}_{eb}"
                        )
                    else:
                        ps = ps2_pool.tile(
                            [128, 512], F32, tag="ps2", name=f"ps2_{c}_{eb}"
                        )
                    for t in range(4 * c):
                        nc.tensor.matmul(
                            ps[:],
                            proj[t][:, eb * 128 : (eb + 1) * 128],
                            wct[:, t * 512 : (t + 1) * 512],
                            start=(t == 0),
                            stop=False,
                        )
                    for j in range(4):
                        n = DIAG_N[j]
                        col = 4 * c * 512 + DIAG_OFF[j]
                        nc.tensor.matmul(
                            ps[:, 128 * j : 512],
                            proj[4 * c + j][:, eb * 128 : (eb + 1) * 128],
                            wct[:, col : col + n],
                            start=(c == 0 and j == 0),
                            stop=(j == 3),
                        )
                    mt = mix_pool.tile(
                        [128, 512], BF16, tag=f"m{eb}_{c}", name=f"mix_{eb}_{c}"
                    )
                    if eb < 2:
                        nc.vector.tensor_copy(mt[:], ps[:])
                    else:
                        nc.scalar.copy(mt[:], ps[:])
                    mix[eb][c] = mt

            def phase3(c):
                for fi in range(4):
                    ft = c * 4 + fi
                    ot = out_pool.tile(
                        [128, D], BF16, tag="out", name=f"out_{ft}"
                    )
                    for dc in range(2):
                        g = 2 * fi + dc
                        ps = psB_pool.tile(
                            [128, 512], F32, tag=f"b{g % 4}", name=f"ps3_{ft}_{dc}"
                        )
                        for eb in range(4):
                            nc.tensor.matmul(
                                ps[:],
                                mix[eb][c][:, fi * 128 : (fi + 1) * 128],
                                wo_all[
                                    :, eb * D + dc * 512 : eb * D + (dc + 1) * 512
                                ],
                                start=(eb == 0),
                                stop=(eb == 3),
                            )
                        osl = ot[:, dc * 512 : (dc + 1) * 512]
                        last = c == FC - 1 and fi == 3 and dc == 1
                        if last:
                            # final cast all on vector: scalar's dispatch
                            # lags ~0.5us behind the last matmul here
                            nc.vector.tensor_copy(osl, ps[:])
                        else:
                            if dc == 0:
                                nc.vector.tensor_copy(osl, ps[:])
                            else:
                                nc.scalar.copy(osl, ps[:])
                            if c == FC - 1 and fi != 3:
                                # last chunk avoids gpsimd entirely: its
                                # SWDGE drain costs ~3us when active at the
                                # end. dc0 on sync (idle by now), dc1 scalar.
                                deng = nc.sync if dc == 0 else nc.scalar
                                deng.dma_start(
                                    part[
                                        ft * 128 : (ft + 1) * 128,
                                        dc * 512 : (dc + 1) * 512,
                                    ],
                                    osl,
                                )
                    if c != FC - 1:
                        nc.gpsimd.dma_start(
                            part[ft * 128 : (ft + 1) * 128, :], ot[:]
                        )
                    elif fi == 3:
                        # ft15: row-halves (2KB contiguous lines) on the two
                        # drain rings -- one DMA instruction per queue
                        nc.sync.dma_start(
                            part[ft * 128 : ft * 128 + 64, :], ot[:64, :]
                        )
                        nc.scalar.dma_start(
                            part[ft * 128 + 64 : (ft + 1) * 128, :], ot[64:, :]
                        )

            phase1(0)
            phase2(0)
            for c in range(1, FC):
                phase1(c)
                phase3(c - 1)
                phase2(c)
            phase3(FC - 1)

    nc.compile()
    _cache["nc"] = nc
    return nc


def _pack_wc_head(wc_h: np.ndarray) -> np.ndarray:
    """tril(Wc[h]) -> [128, 17408]: per f-chunk c, full off-diagonal l-tiles
    0..4c-1 (512 f-cols each) followed by diagonal l-tiles j=0..3 at
    narrowing widths 512-128j (cols 128j..512 of the chunk)."""
    m = np.tril(wc_h)  # [f, l]
    cols = []
    for c in range(FC):
        if c:
            sub = m[c * 512 : (c + 1) * 512, : 4 * c * 128]  # [512 f, 4c*128 l]
            subT = np.ascontiguousarray(sub.T).reshape(4 * c, 128, 512)
            cols.append(subT.transpose(1, 0, 2).reshape(128, 4 * c * 512))
        for j in range(4):
            blk = m[
                c * 512 + 128 * j : (c + 1) * 512,
                (4 * c + j) * 128 : (4 * c + j + 1) * 128,
            ]  # [512-128j f, 128 l]
            cols.append(np.ascontiguousarray(blk.T))
    return np.ascontiguousarray(np.concatenate(cols, axis=1)).astype(
        ml_dtypes.bfloat16
    )


def kernel(x, Wp, bp, Wc, bc, Wo, bo):
    global LAST_EXEC_NS
    x = np.asarray(x, dtype=np.float32)
    Wp = np.asarray(Wp, dtype=np.float32)
    bp = np.asarray(bp, dtype=np.float32)
    Wc = np.asarray(Wc, dtype=np.float32)
    bc = np.asarray(bc, dtype=np.float32)
    Wo = np.asarray(Wo, dtype=np.float32)
    bo = np.asarray(bo, dtype=np.float32)

    nc = _build_program()

    WoT = np.ascontiguousarray(Wo.T)  # [din, dout]
    wc_packed = [_pack_wc_head(Wc[h]) for h in range(H)]
    # Pre-tiled (partition-major) layouts matching the SBUF tiles exactly:
    #   x_pre[b][p, c*4096 + d*512 + l] = x[b, c*512+l, d*128+p]
    #   wp_pre[hp][p, d*512 + e]        = WpT_pair[d*128+p, e]
    #   wo_pre[hp][p, eb*1024 + j]      = WoT_pair[eb*128+p, j]
    x_pre = [
        np.ascontiguousarray(
            x[b].reshape(4, 512, 8, 128).transpose(3, 0, 2, 1).reshape(128, 16384)
        ).astype(ml_dtypes.bfloat16)
        for b in range(B)
    ]
    wp_pre = []
    wo_pre = []
    for hp in range(2):
        h0, h1 = 2 * hp, 2 * hp + 1
        wpT_pair = np.concatenate([Wp[h0].T, Wp[h1].T], axis=1)  # [D, 2E]
        wp_pre.append(
            np.ascontiguousarray(
                wpT_pair.reshape(8, 128, 512).transpose(1, 0, 2).reshape(128, 4096)
            ).astype(ml_dtypes.bfloat16)
        )
        woT_pair = np.concatenate(
            [WoT[h0 * E : (h0 + 1) * E], WoT[h1 * E : (h1 + 1) * E]], axis=0
        )  # [2E, D]
        wo_pre.append(
            np.ascontiguousarray(
                woT_pair.reshape(4, 128, 1024).transpose(1, 0, 2).reshape(128, 4096)
            ).astype(ml_dtypes.bfloat16)
        )

    in_maps = []
    for c in range(8):
        b, hp = c // 2, c % 2
        in_maps.append(
            {
                "xp": x_pre[b],
                "wpp": wp_pre[hp],
                "wc0": wc_packed[2 * hp],
                "wc1": wc_packed[2 * hp + 1],
                "wop": wo_pre[hp],
            }
        )

    res = run_bass_kernel_spmd(
        nc, in_maps, core_ids=list(range(8)), trace=TRACE
    )
    LAST_EXEC_NS = res.exec_time_ns

    # Host: fold all bias terms into one [L, D] matrix.
    # mixed bias = tril-rowsum(Wc)[h,f] * bp[h,e] + bc[h,f]; through Wo:
    rs = np.tril(Wc).sum(axis=2)  # [H, L]
    Wo_hE = Wo.reshape(D, H, E)
    V = np.einsum("he,jhe->hj", bp, Wo_hE)  # [H, D]
    WoSum = Wo_hE.sum(axis=2)  # [D, H]
    bias_total = rs.T @ V + bc.T @ WoSum.T + bo[None, :]  # [L, D]

    out = np.empty((B, L, D), dtype=np.float32)
    for b in range(B):
        out[b] = (
            res.results[2 * b]["part"].astype(np.float32)
            + res.results[2 * b + 1]["part"].astype(np.float32)
            + bias_total
        )
    return out

